# revision 1
# baseline (speedup 1.0000x reference)
"""3-layer GAT on trn2, 8 NeuronCores, edge-parallel with dst-range sharding.

Strategy (per core c, owning dst nodes [c*2500, (c+1)*2500)):
- Edges are bucketed by dst into 20 windows of 125 nodes, padded per-window to a
  multiple of 128 (window sizes are the max over cores so the SPMD program is
  identical on every core).
- Dense phase: each core computes h = act @ W for ITS 2500 nodes, packs DRAM
  "table rows" [h | alpha_src | pad] and AllGathers them so every core holds the
  full 20000-row table; per-node alpha_dst stays local in SBUF.
- Edge phase: per window, one dma_gather pulls the table rows for the window's
  edge sources; per 128-edge chunk the scores e = leaky(as+ad) are computed
  (ad via a TensorE broadcast-transpose + one-hot matmul from the local
  alpha_dst), p = exp(e), and a p-weighted one-hot matmul scatter-accumulates
  [sum p*h | sum p] into PSUM. Window epilogue divides, adds bias, applies
  relu, and transposes the activation for the next layer's dense phase.
"""
import os, sys
for _p in ('/opt/trn_rl_repo', '/root/.axon_site/_ro/trn_rl_repo'):
    if os.path.isdir(_p) and _p not in sys.path:
        sys.path.insert(0, _p)

import numpy as np

import concourse.bacc as bacc
import concourse.tile as tile
from concourse import bass, mybir
from concourse import bass_utils

N = 20000
E = 320000
HID = 64
HEADS = 4
OUT_CH = 64
NEG = 0.2
C = 8
SHARD = N // C          # 2500
WIN = 125               # dst nodes per window
NW = SHARD // WIN       # 20
P = 128

# fin, fout, heads, table row width (f32 elems, 64-multiple >= fout+heads)
LAYERS = [
    dict(fin=64,  fout=256, heads=4, row=320),
    dict(fin=256, fout=256, heads=4, row=320),
    dict(fin=256, fout=64,  heads=1, row=128),
]

AX = mybir.AxisListType
ALU = mybir.AluOpType
ACTF = mybir.ActivationFunctionType
F32 = mybir.dt.float32
I16 = mybir.dt.int16


def _host_prep(edge_index):
    """Returns per-core gather-idx/dstloc arrays and shared window sizes."""
    src = np.asarray(edge_index[0], dtype=np.int64)
    dst = np.asarray(edge_index[1], dtype=np.int64)
    per_core = []   # (srcs, dstloc) per (core, window)
    counts = np.zeros((C, NW), dtype=np.int64)
    for c in range(C):
        m = (dst >= c * SHARD) & (dst < (c + 1) * SHARD)
        es, ed = src[m], dst[m] - c * SHARD
        order = np.argsort(ed, kind='stable')
        es, ed = es[order], ed[order]
        w = ed // WIN
        wins = []
        for wi in range(NW):
            sel = w == wi
            wins.append((es[sel], ed[sel] - wi * WIN))
            counts[c, wi] = sel.sum()
        per_core.append(wins)
    kws = (np.ceil(counts.max(axis=0) / P).astype(np.int64) * P)
    kws = np.maximum(kws, P)
    tot = int(kws.sum())
    idx_all, dl_all = [], []
    for c in range(C):
        idx_mat = np.zeros((16, tot // 16), dtype=np.int16)
        dl_mat = np.full((P, tot // P), float(WIN), dtype=np.float32)
        icol = ccol = 0
        for wi in range(NW):
            kw = int(kws[wi])
            es, dl = per_core[c][wi]
            n = len(es)
            sp = np.zeros(kw, dtype=np.int16)
            dp = np.full(kw, float(WIN), dtype=np.float32)
            sp[:n] = es.astype(np.int16)
            dp[:n] = dl.astype(np.float32)
            idx_mat[:, icol:icol + kw // 16] = sp.reshape(-1, 16).T
            dl_mat[:, ccol:ccol + kw // P] = dp.reshape(-1, P).T
            icol += kw // 16
            ccol += kw // P
        idx_all.append(np.tile(idx_mat, (8, 1)))
        dl_all.append(dl_mat)
    return tuple(int(k) for k in kws), idx_all, dl_all


def build(kws, timing_reps=0):
    """Builds the SPMD bass module. kws: per-window padded edge counts."""
    tot = sum(kws)
    nc = bacc.Bacc("TRN2", target_bir_lowering=False, debug=False, num_devices=C)

    # ---- DRAM I/O ----
    d_xT = nc.dram_tensor("xT_own", [HID, SHARD], F32, kind="ExternalInput")
    d_W = [nc.dram_tensor(f"W{l+1}", [LAYERS[l]['fin'], LAYERS[l]['fout']], F32,
                          kind="ExternalInput") for l in range(3)]
    d_As = [nc.dram_tensor(f"Asr{l+1}", [P, LAYERS[l]['fout']], F32,
                           kind="ExternalInput") for l in range(3)]
    d_Ad = [nc.dram_tensor(f"Adr{l+1}", [P, LAYERS[l]['fout']], F32,
                           kind="ExternalInput") for l in range(3)]
    d_b = [nc.dram_tensor(f"br{l+1}", [P, LAYERS[l]['fout']], F32,
                          kind="ExternalInput") for l in range(3)]
    d_iota = nc.dram_tensor("iota_row", [P, P], F32, kind="ExternalInput")
    d_iotac = nc.dram_tensor("iota_col", [P, 1], F32, kind="ExternalInput")
    d_ident = nc.dram_tensor("ident", [P, P], F32, kind="ExternalInput")
    d_idx = nc.dram_tensor("gat_idx", [P, tot // 16], I16, kind="ExternalInput")
    d_dl = nc.dram_tensor("dstloc", [P, tot // P], F32, kind="ExternalInput")
    d_out = nc.dram_tensor("out", [SHARD, OUT_CH], F32, kind="ExternalOutput")
    if timing_reps:
        d_tok = nc.dram_tensor("tok", [1, 32], F32, kind="ExternalInput")
        d_toko = nc.dram_tensor("tok_out", [1, 32], F32, kind="ExternalOutput")

    tabs = []
    for l, cfg in enumerate(LAYERS):
        s = nc.dram_tensor(f"tab{l+1}s", [SHARD, cfg['row']], F32)
        f = nc.dram_tensor(f"tab{l+1}f", [N, cfg['row']], F32, addr_space="Shared")
        tabs.append((s, f))

    with tile.TileContext(nc) as tc:
        with tc.tile_pool(name="const", bufs=1) as cp, \
             tc.tile_pool(name="rowp", bufs=2) as rowp, \
             tc.tile_pool(name="gp", bufs=2) as gp, \
             tc.tile_pool(name="sp", bufs=3) as sp, \
             tc.tile_pool(name="rp", bufs=3) as rp, \
             tc.tile_pool(name="op", bufs=2) as op_, \
             tc.tile_pool(name="ps", bufs=1, space="PSUM") as pp:

            # ---- persistent SBUF ----
            iota = cp.tile([P, P], F32)
            iotac = cp.tile([P, 1], F32)
            ident = cp.tile([P, P], F32)
            nc.sync.dma_start(iota[:], d_iota[:, :])
            nc.sync.dma_start(iotac[:], d_iotac[:, :])
            nc.sync.dma_start(ident[:], d_ident[:, :])
            idx_sb = cp.tile([P, tot // 16], I16)
            dl_sb = cp.tile([P, tot // P], F32)
            nc.sync.dma_start(idx_sb[:], d_idx[:, :])
            nc.sync.dma_start(dl_sb[:], d_dl[:, :])
            xT = cp.tile([HID, SHARD], F32)
            nc.sync.dma_start(xT[:], d_xT[:, :])
            Wt, Ast, Adt, bt = [], [], [], []
            for l, cfg in enumerate(LAYERS):
                fin, fout = cfg['fin'], cfg['fout']
                chunks = []
                for kc in range(0, fin, P):
                    ke = min(kc + P, fin)
                    t = cp.tile([ke - kc, fout], F32, tag=f"W{l}_{kc}")
                    nc.sync.dma_start(t[:], d_W[l][kc:ke, :])
                    chunks.append(t)
                Wt.append(chunks)
                a = cp.tile([P, fout], F32, tag=f"As{l}")
                nc.sync.dma_start(a[:], d_As[l][:, :])
                Ast.append(a)
                a = cp.tile([P, fout], F32, tag=f"Ad{l}")
                nc.sync.dma_start(a[:], d_Ad[l][:, :])
                Adt.append(a)
                a = cp.tile([P, fout], F32, tag=f"b{l}")
                nc.sync.dma_start(a[:], d_b[l][:, :])
                bt.append(a)
            actT = {1: [cp.tile([P, SHARD], F32, tag=f"actT1_{j}", name=f"actT1_{j}")
                        for j in range(2)],
                    2: [cp.tile([P, SHARD], F32, tag=f"actT2_{j}", name=f"actT2_{j}")
                        for j in range(2)]}
            ado = [cp.tile([P, NW * LAYERS[l]['heads']], F32, tag=f"ado{l}",
                           name=f"ado{l}") for l in range(3)]

            def dense_phase(l, actT_in):
                cfg = LAYERS[l]
                fin, fout, heads, row = cfg['fin'], cfg['fout'], cfg['heads'], cfg['row']
                tab_s = tabs[l][0]
                nchunks = len(Wt[l])
                for w in range(NW):
                    ph = pp.tile([WIN, fout], F32, tag="ph")
                    for kc in range(nchunks):
                        nc.tensor.matmul(
                            ph[:, :], lhsT=actT_in[kc][:, w * WIN:(w + 1) * WIN],
                            rhs=Wt[l][kc][:], start=(kc == 0), stop=(kc == nchunks - 1))
                    row_t = rowp.tile([P, row], F32, tag="row")
                    nc.vector.tensor_copy(row_t[:WIN, 0:fout], ph[:, :])
                    ts = rowp.tile([P, 256], F32, tag="ts")
                    nc.vector.tensor_tensor(ts[:WIN, :fout], ph[:, :], Ast[l][:WIN, :],
                                            op=ALU.mult)
                    nc.vector.tensor_reduce(
                        row_t[:WIN, fout:fout + heads],
                        ts[:WIN, :fout].rearrange("p (h d) -> p h d", h=heads),
                        axis=AX.X, op=ALU.add)
                    nc.vector.tensor_tensor(ts[:WIN, :fout], ph[:, :], Adt[l][:WIN, :],
                                            op=ALU.mult)
                    nc.vector.tensor_reduce(
                        ado[l][:WIN, w * heads:(w + 1) * heads],
                        ts[:WIN, :fout].rearrange("p (h d) -> p h d", h=heads),
                        axis=AX.X, op=ALU.add)
                    if row > fout + heads:
                        nc.vector.memset(row_t[:, fout + heads:row], 0.0)
                    nc.sync.dma_start(tab_s[w * WIN:(w + 1) * WIN, :], row_t[:WIN, :])

            def allgather(l):
                tab_s, tab_f = tabs[l]
                if timing_reps:
                    nshard = C if os.environ.get("GAT_AG_MODE", "mock8") == "mock8" else 1
                    for s in range(nshard):
                        nc.sync.dma_start(tab_f[s * SHARD:(s + 1) * SHARD, :],
                                          tab_s[:, :])
                else:
                    nc.gpsimd.collective_compute(
                        "AllGather", ALU.bypass,
                        replica_groups=[list(range(C))],
                        ins=[tab_s[:, :]], outs=[tab_f[:, :]])

            def edge_phase(l, actT_next):
                cfg = LAYERS[l]
                fout, heads, row = cfg['fout'], cfg['heads'], cfg['row']
                dh = fout // heads
                ncols = fout + heads
                tab_f = tabs[l][1]
                icol = ccol = 0
                for w in range(NW):
                    kw = kws[w]
                    tw = kw // P
                    gw = gp.tile([P, tw * row], F32, tag="gw")
                    gw3 = gw[:].rearrange("p (t e) -> p t e", e=row)
                    nc.gpsimd.dma_gather(
                        gw3, tab_f[:, :],
                        idx_sb[:, icol:icol + kw // 16],
                        kw, kw, row, single_packet=False)
                    psw = pp.tile([WIN, ncols], F32, tag="psw", bufs=2)
                    ad_win = ado[l][:WIN, w * heads:(w + 1) * heads]
                    for t in range(tw):
                        o = t * row
                        dcol = dl_sb[:, ccol + t:ccol + t + 1]
                        # ad per edge: broadcast dstloc along free, transpose,
                        # compare to per-partition iota -> onehotT, small matmul
                        pb = pp.tile([P, P], F32, tag="pb", bufs=2)
                        nc.tensor.transpose(pb[:], dcol.to_broadcast([P, P]), ident[:])
                        ohT = sp.tile([WIN, P], F32, tag="ohT")
                        nc.vector.tensor_scalar(ohT[:], pb[:WIN, :], iotac[:WIN, :],
                                                None, op0=ALU.is_equal)
                        pads = pp.tile([P, heads], F32, tag="pads", bufs=1)
                        nc.tensor.matmul(pads[:], lhsT=ohT[:], rhs=ad_win,
                                         start=True, stop=True)
                        st = sp.tile([P, heads], F32, tag="st")
                        nc.vector.tensor_tensor(st[:], gw[:, o + fout:o + fout + heads],
                                                pads[:], op=ALU.add)
                        lt = sp.tile([P, heads], F32, tag="lt")
                        nc.vector.tensor_scalar(lt[:], st[:], NEG, None, op0=ALU.mult)
                        nc.vector.tensor_tensor(lt[:], lt[:], st[:], op=ALU.max)
                        rhs_t = rp.tile([P, ncols], F32, tag="rhs")
                        nc.scalar.activation(rhs_t[:, fout:fout + heads], lt[:],
                                             ACTF.Exp)
                        wgt = os.environ.get("GAT_WGT", "split")
                        if wgt == "dve":
                            nc.vector.tensor_tensor(
                                rhs_t[:, 0:fout].rearrange("p (h d) -> p h d", d=dh),
                                gw[:, o:o + fout].rearrange("p (h d) -> p h d", d=dh),
                                rhs_t[:, fout:fout + heads].broadcast_to(
                                    (P, heads, dh)),
                                op=ALU.mult)
                        else:
                            for h in range(heads):
                                if wgt == "split" and h % 2 == 0:
                                    nc.vector.tensor_scalar(
                                        rhs_t[:, h * dh:(h + 1) * dh],
                                        gw[:, o + h * dh:o + (h + 1) * dh],
                                        rhs_t[:, fout + h:fout + h + 1], None,
                                        op0=ALU.mult)
                                else:
                                    nc.scalar.activation(
                                        rhs_t[:, h * dh:(h + 1) * dh],
                                        gw[:, o + h * dh:o + (h + 1) * dh],
                                        ACTF.Copy,
                                        scale=rhs_t[:, fout + h:fout + h + 1])
                        oh = sp.tile([P, WIN], F32, tag="oh")
                        nc.vector.tensor_scalar(oh[:], iota[:, :WIN], dcol, None,
                                                op0=ALU.is_equal)
                        nc.tensor.matmul(psw[:], lhsT=oh[:], rhs=rhs_t[:],
                                         start=(t == 0), stop=(t == tw - 1))
                    # window epilogue
                    den = sp.tile([WIN, heads], F32, tag="den")
                    nc.vector.tensor_scalar(den[:], psw[:, fout:fout + heads], 1e-16,
                                            None, op0=ALU.add)
                    rec = sp.tile([WIN, heads], F32, tag="rec")
                    nc.vector.reciprocal(rec[:], den[:])
                    orow = op_.tile([P, fout], F32, tag="orow")
                    for h in range(heads):
                        nc.scalar.activation(orow[:WIN, h * dh:(h + 1) * dh],
                                             psw[:, h * dh:(h + 1) * dh],
                                             ACTF.Copy, scale=rec[:, h:h + 1])
                    nc.vector.tensor_tensor(orow[:WIN, :], orow[:WIN, :],
                                            bt[l][:WIN, :], op=ALU.add)
                    if l < 2:
                        nc.vector.tensor_scalar(orow[:WIN, :], orow[:WIN, :], 0.0,
                                                None, op0=ALU.max)
                        for j in range(fout // P):
                            pt = pp.tile([P, WIN], F32, tag="pt", bufs=1)
                            nc.tensor.transpose(pt[:], orow[:WIN, j * P:(j + 1) * P],
                                                ident[:WIN, :WIN])
                            nc.vector.tensor_copy(
                                actT_next[j][:, w * WIN:(w + 1) * WIN], pt[:])
                    else:
                        nc.sync.dma_start(d_out[w * WIN:(w + 1) * WIN, :],
                                          orow[:WIN, :fout])
                    icol += kw // 16
                    ccol += kw // P

            def body():
                stages = int(os.environ.get("GAT_STAGES", "9"))  # 9 = full network
                dense_phase(0, [xT])
                if stages >= 2:
                    allgather(0)
                if stages >= 3:
                    edge_phase(0, actT[1])
                if stages >= 4:
                    dense_phase(1, actT[1])
                    allgather(1)
                if stages >= 5:
                    edge_phase(1, actT[2])
                if stages >= 6:
                    dense_phase(2, actT[2])
                    allgather(2)
                if stages >= 7:
                    edge_phase(2, None)
                if stages < 7:
                    z = op_.tile([WIN, OUT_CH], F32, tag="z", name="z")
                    nc.vector.memset(z[:], 0.0)
                    for w in range(NW):
                        nc.sync.dma_start(d_out[w * WIN:(w + 1) * WIN, :], z[:])

            if timing_reps:
                tk = cp.tile([1, 32], F32)
                nc.sync.dma_start(tk[:], d_tok[:, :])
                if timing_reps == 1:
                    body()
                else:
                    with tc.For_i(0, timing_reps, 1):
                        body()
                nc.sync.dma_start(d_toko[:, :], tk[:])
            else:
                body()

    nc.compile()
    return nc


def _host_inputs(x, edge_index, W1, a1s, a1d, b1, W2, a2s, a2d, b2, W3, a3s, a3d, b3):
    kws, idx_all, dl_all = _host_prep(edge_index)
    x = np.asarray(x, dtype=np.float32)
    Ws = [np.asarray(W1, np.float32), np.asarray(W2, np.float32),
          np.asarray(W3, np.float32)]
    As = [np.asarray(a1s, np.float32), np.asarray(a2s, np.float32),
          np.asarray(a3s, np.float32)]
    Ad = [np.asarray(a1d, np.float32), np.asarray(a2d, np.float32),
          np.asarray(a3d, np.float32)]
    bs = [np.asarray(b1, np.float32), np.asarray(b2, np.float32),
          np.asarray(b3, np.float32)]
    shared = {}
    for l in range(3):
        fout = LAYERS[l]['fout']
        shared[f"W{l+1}"] = Ws[l]
        shared[f"Asr{l+1}"] = np.tile(As[l].reshape(1, fout), (P, 1))
        shared[f"Adr{l+1}"] = np.tile(Ad[l].reshape(1, fout), (P, 1))
        shared[f"br{l+1}"] = np.tile(bs[l].reshape(1, fout), (P, 1))
    shared["iota_row"] = np.tile(np.arange(P, dtype=np.float32).reshape(1, P), (P, 1))
    shared["iota_col"] = np.arange(P, dtype=np.float32).reshape(P, 1)
    shared["ident"] = np.eye(P, dtype=np.float32)
    in_maps = []
    for c in range(C):
        m = dict(shared)
        m["xT_own"] = np.ascontiguousarray(x[c * SHARD:(c + 1) * SHARD].T)
        m["gat_idx"] = idx_all[c]
        m["dstloc"] = dl_all[c]
        in_maps.append(m)
    return kws, in_maps


_CACHE = {}


def kernel(**inputs) -> np.ndarray:
    kws, in_maps = _host_inputs(**inputs)
    if kws not in _CACHE:
        _CACHE[kws] = build(kws)
    nc = _CACHE[kws]
    last = None
    for _attempt in range(2):
        try:
            res = bass_utils.run_bass_kernel_spmd(
                nc, in_maps, core_ids=list(range(C)), trace=False)
            return np.concatenate(
                [res.results[c]["out"] for c in range(C)], axis=0)
        except Exception as e:  # rare transient device-mesh hiccups: retry once
            last = e
    raise last



# revision 2
# speedup vs baseline: 1.7290x; 1.7290x over previous
"""3-layer GAT on trn2, 8 NeuronCores, edge-parallel with dst-range sharding. v2.

Strategy (per core c, owning dst nodes [c*2500, (c+1)*2500)):
- Edges bucketed by dst into 20 windows of 125 nodes, padded per-window to a
  multiple of 128 (window sizes shared across cores -> identical SPMD program).
- Dense phase: h_aug = act @ [W | W@a_s | W@a_d] in one fp16 matmul per window
  (alpha projections folded into the weights on the host). Table rows hold
  [h+bias | u=exp(as) | v=exp(0.2*as)] in fp16; U=exp(ad), V=exp(0.2*ad) stay
  in SBUF per dst window. Bias folds into h exactly (softmax weights sum to 1).
- AllGather shares the fp16 table (mock local copies in timing builds).
- Edge phase: exp(leaky(as+ad)) == max(u*U, v*V), so no per-edge exp is
  needed. Per 128-edge chunk: one-hot oh0 (DVE is_equal), PE transpose ->
  ohT, pads = ohT @ [U|V] (PE) gives per-edge U,V; per-window batched DVE ops
  form p = max(u*U, v*V); p-weighted gathered h forms the rhs of a one-hot
  scatter matmul accumulating [sum p*h | sum p] into PSUM. Epilogue divides,
  relu, transposes activations for the next dense phase.
"""
import os, sys
for _p in ('/opt/trn_rl_repo', '/root/.axon_site/_ro/trn_rl_repo'):
    if os.path.isdir(_p) and _p not in sys.path:
        sys.path.insert(0, _p)

import numpy as np

import concourse.bacc as bacc
import concourse.tile as tile
from concourse import bass, mybir
from concourse import bass_utils

N = 20000
E = 320000
HID = 64
HEADS = 4
OUT_CH = 64
NEG = 0.2
C = 8
SHARD = N // C          # 2500
WIN = 125               # dst nodes per window
NW = SHARD // WIN       # 20
P = 128

# fin, fout, heads, table row width (fp16 elems, 128-multiple >= fout+2*heads)
LAYERS = [
    dict(fin=64,  fout=256, heads=4, row=384),
    dict(fin=256, fout=256, heads=4, row=384),
    dict(fin=256, fout=64,  heads=1, row=128),
]

AX = mybir.AxisListType
ALU = mybir.AluOpType
ACTF = mybir.ActivationFunctionType
F32 = mybir.dt.float32
F16 = mybir.dt.float16
I16 = mybir.dt.int16


def _host_prep(edge_index):
    """Returns per-core gather-idx/dstloc arrays and shared window sizes."""
    src = np.asarray(edge_index[0], dtype=np.int64)
    dst = np.asarray(edge_index[1], dtype=np.int64)
    per_core = []   # (srcs, dstloc) per (core, window)
    counts = np.zeros((C, NW), dtype=np.int64)
    for c in range(C):
        m = (dst >= c * SHARD) & (dst < (c + 1) * SHARD)
        es, ed = src[m], dst[m] - c * SHARD
        order = np.argsort(ed, kind='stable')
        es, ed = es[order], ed[order]
        w = ed // WIN
        wins = []
        for wi in range(NW):
            sel = w == wi
            wins.append((es[sel], ed[sel] - wi * WIN))
            counts[c, wi] = sel.sum()
        per_core.append(wins)
    kws = (np.ceil(counts.max(axis=0) / P).astype(np.int64) * P)
    kws = np.maximum(kws, P)
    tot = int(kws.sum())
    idx_all, dl_all = [], []
    for c in range(C):
        idx_mat = np.zeros((16, tot // 16), dtype=np.int16)
        dl_mat = np.full((P, tot // P), float(WIN), dtype=np.float32)
        icol = ccol = 0
        for wi in range(NW):
            kw = int(kws[wi])
            es, dl = per_core[c][wi]
            n = len(es)
            sp = np.zeros(kw, dtype=np.int16)
            dp = np.full(kw, float(WIN), dtype=np.float32)
            sp[:n] = es.astype(np.int16)
            dp[:n] = dl.astype(np.float32)
            idx_mat[:, icol:icol + kw // 16] = sp.reshape(-1, 16).T
            dl_mat[:, ccol:ccol + kw // P] = dp.reshape(-1, P).T
            icol += kw // 16
            ccol += kw // P
        idx_all.append(np.tile(idx_mat, (8, 1)))
        dl_all.append(dl_mat)
    return tuple(int(k) for k in kws), idx_all, dl_all


def build(kws, timing_reps=0):
    """Builds the SPMD bass module. kws: per-window padded edge counts."""
    tot = sum(kws)
    nq = int(os.environ.get("GAT_QUEUES", "4"))
    nc = bacc.Bacc("TRN2", target_bir_lowering=False, debug=False, num_devices=C,
                   num_swdge_queues=nq)

    # ---- DRAM I/O ----
    d_xT = nc.dram_tensor("xT_own", [HID, SHARD], F16, kind="ExternalInput")
    d_W = [nc.dram_tensor(f"Waug{l+1}",
                          [LAYERS[l]['fin'], LAYERS[l]['fout'] + 2 * LAYERS[l]['heads']],
                          F16, kind="ExternalInput") for l in range(3)]
    d_b = [nc.dram_tensor(f"br{l+1}", [P, LAYERS[l]['fout']], F16,
                          kind="ExternalInput") for l in range(3)]
    d_iota = nc.dram_tensor("iota16", [P, P], F16, kind="ExternalInput")
    d_ident = nc.dram_tensor("ident16", [P, P], F16, kind="ExternalInput")
    d_idx = nc.dram_tensor("gat_idx", [P, tot // 16], I16, kind="ExternalInput")
    d_dl = nc.dram_tensor("dstloc", [P, tot // P], F32, kind="ExternalInput")
    d_out = nc.dram_tensor("out", [SHARD, OUT_CH], F32, kind="ExternalOutput")
    if timing_reps:
        d_tok = nc.dram_tensor("tok", [1, 32], F32, kind="ExternalInput")
        d_toko = nc.dram_tensor("tok_out", [1, 32], F32, kind="ExternalOutput")

    tabs = []
    for l, cfg in enumerate(LAYERS):
        s = nc.dram_tensor(f"tab{l+1}s", [SHARD, cfg['row']], F16)
        f = nc.dram_tensor(f"tab{l+1}f", [N, cfg['row']], F16, addr_space="Shared")
        tabs.append((s, f))

    with tile.TileContext(nc) as tc:
        with tc.tile_pool(name="const", bufs=1) as cp, \
             tc.tile_pool(name="rowp", bufs=2) as rowp, \
             tc.tile_pool(name="gp", bufs=2) as gp, \
             tc.tile_pool(name="sp", bufs=3) as sp, \
             tc.tile_pool(name="rp", bufs=2) as rp, \
             tc.tile_pool(name="op", bufs=2) as op_, \
             tc.tile_pool(name="ps", bufs=1, space="PSUM") as pp:

            # ---- persistent SBUF ----
            iota16 = cp.tile([P, P], F16)
            ident16 = cp.tile([P, P], F16)
            nc.sync.dma_start(iota16[:], d_iota[:, :])
            nc.sync.dma_start(ident16[:], d_ident[:, :])
            idx_sb = cp.tile([P, tot // 16], I16)
            dl_sb = cp.tile([P, tot // P], F32)
            nc.sync.dma_start(idx_sb[:], d_idx[:, :])
            nc.sync.dma_start(dl_sb[:], d_dl[:, :])
            xT = cp.tile([HID, SHARD], F16)
            nc.sync.dma_start(xT[:], d_xT[:, :])
            Wt, bt, UVt = [], [], []
            for l, cfg in enumerate(LAYERS):
                fin, fout, heads = cfg['fin'], cfg['fout'], cfg['heads']
                waug_cols = fout + 2 * heads
                chunks = []
                for kc in range(0, fin, P):
                    ke = min(kc + P, fin)
                    t = cp.tile([ke - kc, waug_cols], F16, tag=f"W{l}_{kc}")
                    nc.sync.dma_start(t[:], d_W[l][kc:ke, :])
                    chunks.append(t)
                Wt.append(chunks)
                b = cp.tile([P, fout], F16, tag=f"b{l}")
                nc.sync.dma_start(b[:], d_b[l][:, :])
                bt.append(b)
                uv = cp.tile([P, NW * 2 * heads], F16, tag=f"UV{l}", name=f"UV{l}")
                UVt.append(uv)
            actT = {1: [cp.tile([P, SHARD], F16, tag=f"actT1_{j}", name=f"actT1_{j}")
                        for j in range(2)],
                    2: [cp.tile([P, SHARD], F16, tag=f"actT2_{j}", name=f"actT2_{j}")
                        for j in range(2)]}

            def dense_phase(l, actT_in):
                cfg = LAYERS[l]
                fin, fout, heads, row = cfg['fin'], cfg['fout'], cfg['heads'], cfg['row']
                h2 = 2 * heads
                tab_s = tabs[l][0]
                nchunks = len(Wt[l])
                # zero the U/V tile once per layer pass (pad dst rows must be 0;
                # rows < WIN are overwritten by the per-window exps below)
                nc.vector.memset(UVt[l][:, :], 0.0)
                for w in range(NW):
                    ph = pp.tile([WIN, fout + h2], F32, tag="ph", bufs=2)
                    for kc in range(nchunks):
                        nc.tensor.matmul(
                            ph[:, :], lhsT=actT_in[kc][:, w * WIN:(w + 1) * WIN],
                            rhs=Wt[l][kc][:], start=(kc == 0), stop=(kc == nchunks - 1))
                    row_t = rowp.tile([P, row], F16, tag="row")
                    # h + bias (bias folds through the softmax average exactly)
                    nc.vector.tensor_tensor(row_t[:WIN, 0:fout], ph[:, 0:fout],
                                            bt[l][:WIN, :], op=ALU.add)
                    # u = exp(as), v = exp(0.2*as) ride along in the table row
                    nc.scalar.activation(row_t[:WIN, fout:fout + heads],
                                         ph[:, fout:fout + heads], ACTF.Exp)
                    nc.scalar.activation(row_t[:WIN, fout + heads:fout + h2],
                                         ph[:, fout:fout + heads], ACTF.Exp, scale=NEG)
                    if row > fout + h2:
                        nc.vector.memset(row_t[:WIN, fout + h2:row], 0.0)
                    # U = exp(ad), V = exp(0.2*ad) stay local per dst window
                    nc.scalar.activation(UVt[l][:WIN, w * h2:w * h2 + heads],
                                         ph[:, fout + heads:fout + h2], ACTF.Exp)
                    nc.scalar.activation(UVt[l][:WIN, w * h2 + heads:(w + 1) * h2],
                                         ph[:, fout + heads:fout + h2], ACTF.Exp,
                                         scale=NEG)
                    nc.sync.dma_start(tab_s[w * WIN:(w + 1) * WIN, :], row_t[:WIN, :])

            def allgather(l):
                tab_s, tab_f = tabs[l]
                if timing_reps:
                    nshard = C if os.environ.get("GAT_AG_MODE", "mock8") == "mock8" else 1
                    for s in range(nshard):
                        nc.sync.dma_start(tab_f[s * SHARD:(s + 1) * SHARD, :],
                                          tab_s[:, :])
                else:
                    nc.gpsimd.collective_compute(
                        "AllGather", ALU.bypass,
                        replica_groups=[list(range(C))],
                        ins=[tab_s[:, :]], outs=[tab_f[:, :]])

            def edge_phase(l, actT_next):
                cfg = LAYERS[l]
                fout, heads, row = cfg['fout'], cfg['heads'], cfg['row']
                h2 = 2 * heads
                dh = fout // heads
                ncols = fout + heads
                tab_f = tabs[l][1]
                UV = UVt[l]
                mode = os.environ.get("GAT_EDGE_MODE", "full")
                if l == 2:
                    mode = os.environ.get("GAT_L3MODE", mode)
                icol = ccol = 0
                for w in range(NW):
                    kw = kws[w]
                    tw = kw // P
                    gw = gp.tile([P, tw * row], F16, tag="gw")
                    gw3 = gw[:].rearrange("p (t e) -> p t e", e=row)
                    gsplit = int(os.environ.get("GAT_GSPLIT", "4"))
                    sp_flag = os.environ.get("GAT_SP", "0") == "1"
                    nsub = min(gsplit, tw)
                    qbase = (w * nsub) % nq if nq > 1 else 0
                    tpos = 0
                    for s_i in range(nsub):
                        tcnt = (tw + nsub - 1 - s_i) // nsub
                        if tcnt == 0:
                            continue
                        nidx = tcnt * P
                        nc.gpsimd.dma_gather(
                            gw3[:, tpos:tpos + tcnt, :], tab_f[:, :],
                            idx_sb[:, icol + tpos * P // 16:
                                   icol + (tpos + tcnt) * P // 16],
                            nidx, nidx, row, single_packet=sp_flag,
                            queue_num=(qbase + s_i) % nq)
                        tpos += tcnt
                    if mode == "gather":
                        # timing-only: consume the gather, skip all edge math
                        zz = sp.tile([P, 1], F16, tag="zz")
                        nc.vector.tensor_copy(zz[:], gw[:, 0:1])
                        icol += kw // 16
                        ccol += kw // P
                        continue
                    # static one-hots + per-edge U,V via transpose matmul
                    ohall = sp.tile([P, tw * P], F16, tag="ohall", bufs=2)
                    pads = pp.tile([P, tw * h2], F32, tag="pads", bufs=1)
                    uvw = UV[:, w * h2:(w + 1) * h2]
                    for t in range(tw):
                        nc.vector.tensor_scalar(
                            ohall[:, t * P:(t + 1) * P], iota16[:],
                            dl_sb[:, ccol + t:ccol + t + 1], None, op0=ALU.is_equal)
                    if mode == "oh":
                        zz = sp.tile([P, 1], F16, tag="zz")
                        nc.vector.tensor_copy(zz[:], ohall[:, 0:1])
                        icol += kw // 16
                        ccol += kw // P
                        continue
                    for t in range(tw):
                        if mode in ("full", "noweight"):
                            ohT_ps = pp.tile([P, P], F16, tag="ohT", bufs=2)
                            nc.tensor.transpose(ohT_ps[:],
                                                ohall[:, t * P:(t + 1) * P],
                                                ident16[:])
                            ohT_sb = sp.tile([P, P], F16, tag="ohTs")
                            nc.scalar.activation(ohT_sb[:], ohT_ps[:], ACTF.Copy)
                            nc.tensor.matmul(pads[:, t * h2:(t + 1) * h2],
                                             lhsT=ohT_sb[:],
                                             rhs=uvw, start=True, stop=True)
                    rhs_big = rp.tile([P, tw * ncols], F16, tag="rhs")
                    pw32 = sp.tile([P, tw * heads], F32, tag="pw32")
                    gwv = gw[:].rearrange("p (t e) -> p t e", e=row)
                    pw3 = pw32[:].rearrange("p (t h) -> p t h", h=heads)
                    rh3 = rhs_big[:].rearrange("p (t c) -> p t c", c=ncols)
                    if mode in ("full", "noweight"):
                        # p = max(u*U, v*V), batched over the whole window
                        padsS = sp.tile([P, tw * h2], F16, tag="padsS")
                        nc.scalar.activation(padsS[:], pads[:], ACTF.Copy)
                        tmpu = sp.tile([P, tw * heads], F16, tag="tmpu")
                        tmpv = sp.tile([P, tw * heads], F16, tag="tmpv")
                        pdv = padsS[:].rearrange("p (t c) -> p t c", c=h2)
                        tm3u = tmpu[:].rearrange("p (t h) -> p t h", h=heads)
                        tm3v = tmpv[:].rearrange("p (t h) -> p t h", h=heads)
                        nc.vector.tensor_tensor(tm3u, gwv[:, :, fout:fout + heads],
                                                pdv[:, :, 0:heads], op=ALU.mult)
                        nc.vector.tensor_tensor(tm3v,
                                                gwv[:, :, fout + heads:fout + h2],
                                                pdv[:, :, heads:h2], op=ALU.mult)
                        nc.vector.tensor_tensor(pw3, tm3u, tm3v, op=ALU.max)
                    else:
                        # timing-only knockout: p := gathered u column
                        nc.vector.tensor_copy(pw3, gwv[:, :, fout:fout + heads])
                    nc.vector.tensor_copy(rh3[:, :, fout:fout + heads], pw3)
                    psw = pp.tile([WIN, ncols], F32, tag="psw", bufs=2)
                    for t in range(tw):
                        o = t * ncols
                        if mode in ("full", "noadv") and mode != "mm":
                            for hd in range(heads):
                                pcol = pw32[:, t * heads + hd:t * heads + hd + 1]
                                nc.vector.tensor_scalar(
                                    rhs_big[:, o + hd * dh:o + (hd + 1) * dh],
                                    gw[:, t * row + hd * dh:t * row + (hd + 1) * dh],
                                    pcol, None, op0=ALU.mult)
                        nc.tensor.matmul(psw[:, :], lhsT=ohall[:, t * P:t * P + WIN],
                                         rhs=rhs_big[:, o:o + ncols],
                                         start=(t == 0), stop=(t == tw - 1))
                    # window epilogue
                    den = sp.tile([WIN, heads], F32, tag="den")
                    nc.vector.tensor_scalar(den[:], psw[:, fout:fout + heads], 1e-16,
                                            None, op0=ALU.add)
                    rec = sp.tile([WIN, heads], F32, tag="rec")
                    nc.vector.reciprocal(rec[:], den[:])
                    if l < 2:
                        orow = op_.tile([P, fout], F16, tag="orow")
                        for hd in range(heads):
                            nc.scalar.activation(orow[:WIN, hd * dh:(hd + 1) * dh],
                                                 psw[:, hd * dh:(hd + 1) * dh],
                                                 ACTF.Copy, scale=rec[:, hd:hd + 1])
                        nc.vector.tensor_scalar(orow[:WIN, :], orow[:WIN, :], 0.0,
                                                None, op0=ALU.max)
                        for j in range(fout // P):
                            pt = pp.tile([P, WIN], F16, tag="pt", bufs=1)
                            nc.tensor.transpose(pt[:], orow[:WIN, j * P:(j + 1) * P],
                                                ident16[:WIN, :WIN])
                            nc.vector.tensor_copy(
                                actT_next[j][:, w * WIN:(w + 1) * WIN], pt[:])
                    else:
                        orow = op_.tile([P, fout], F32, tag="orow3")
                        nc.scalar.activation(orow[:WIN, :], psw[:, 0:fout],
                                             ACTF.Copy, scale=rec[:, 0:1])
                        nc.sync.dma_start(d_out[w * WIN:(w + 1) * WIN, :],
                                          orow[:WIN, :fout])
                    icol += kw // 16
                    ccol += kw // P

            def body():
                stages = int(os.environ.get("GAT_STAGES", "9"))  # 9 = full network
                dense_phase(0, [xT])
                if stages >= 2:
                    allgather(0)
                if stages >= 3:
                    edge_phase(0, actT[1])
                if stages >= 4:
                    dense_phase(1, actT[1])
                    allgather(1)
                if stages >= 5:
                    edge_phase(1, actT[2])
                if stages >= 6:
                    dense_phase(2, actT[2])
                    allgather(2)
                if stages >= 7:
                    edge_phase(2, None)
                if stages < 7:
                    z = op_.tile([WIN, OUT_CH], F32, tag="z", name="z")
                    nc.vector.memset(z[:], 0.0)
                    for w in range(NW):
                        nc.sync.dma_start(d_out[w * WIN:(w + 1) * WIN, :], z[:])

            if timing_reps:
                tk = cp.tile([1, 32], F32)
                nc.sync.dma_start(tk[:], d_tok[:, :])
                if timing_reps == 1:
                    body()
                else:
                    with tc.For_i(0, timing_reps, 1):
                        body()
                nc.sync.dma_start(d_toko[:, :], tk[:])
            else:
                body()

    nc.compile()
    return nc


def _host_inputs(x, edge_index, W1, a1s, a1d, b1, W2, a2s, a2d, b2, W3, a3s, a3d, b3):
    kws, idx_all, dl_all = _host_prep(edge_index)
    x = np.asarray(x, dtype=np.float32)
    Ws = [np.asarray(W1, np.float32), np.asarray(W2, np.float32),
          np.asarray(W3, np.float32)]
    As = [np.asarray(a1s, np.float32), np.asarray(a2s, np.float32),
          np.asarray(a3s, np.float32)]
    Ad = [np.asarray(a1d, np.float32), np.asarray(a2d, np.float32),
          np.asarray(a3d, np.float32)]
    bs = [np.asarray(b1, np.float32), np.asarray(b2, np.float32),
          np.asarray(b3, np.float32)]
    shared = {}
    for l in range(3):
        cfg = LAYERS[l]
        fin, fout, heads = cfg['fin'], cfg['fout'], cfg['heads']
        dh = fout // heads
        W = Ws[l]
        Was = np.einsum('fhd,hd->fh', W.reshape(fin, heads, dh), As[l])
        Wad = np.einsum('fhd,hd->fh', W.reshape(fin, heads, dh), Ad[l])
        shared[f"Waug{l+1}"] = np.concatenate([W, Was, Wad], axis=1).astype(np.float16)
        shared[f"br{l+1}"] = np.tile(bs[l].reshape(1, fout), (P, 1)).astype(np.float16)
    shared["iota16"] = np.tile(np.arange(P, dtype=np.float16).reshape(1, P), (P, 1))
    shared["ident16"] = np.eye(P, dtype=np.float16)
    in_maps = []
    for c in range(C):
        m = dict(shared)
        m["xT_own"] = np.ascontiguousarray(
            x[c * SHARD:(c + 1) * SHARD].T).astype(np.float16)
        m["gat_idx"] = idx_all[c]
        m["dstloc"] = dl_all[c]
        in_maps.append(m)
    return kws, in_maps


_CACHE = {}


def kernel(**inputs) -> np.ndarray:
    kws, in_maps = _host_inputs(**inputs)
    if kws not in _CACHE:
        _CACHE[kws] = build(kws)
    nc = _CACHE[kws]
    last = None
    for _attempt in range(2):
        try:
            res = bass_utils.run_bass_kernel_spmd(
                nc, in_maps, core_ids=list(range(C)), trace=False)
            return np.concatenate(
                [res.results[c]["out"] for c in range(C)], axis=0)
        except Exception as e:  # rare transient device-mesh hiccups: retry once
            last = e
    raise last
